# revision 14
# baseline (speedup 1.0000x reference)
"""Causal GQA self-attention (B=2, T=2048, C=2048, 16 heads / 4 KV groups,
head_size=128, RoPE) on 8 Trainium2 NeuronCores.

Sharding: tensor-parallel over the 4 KV groups x data-parallel over the 2
batch elements -> 8 cores, core = b*4 + g. Each core computes its group's
QKV projection, RoPE, causal SDPA for the group's 4 query heads, and the
partial output projection (w_proj input-dim shard). The proj partials are
reduced on the host (equivalent of the post-proj all-reduce).

All matmuls run in bf16 with fp32 PSUM accumulation. Inputs are transposed
and cast to bf16 on the host so every DMA is a contiguous, layout-perfect
load (contraction dims land on SBUF partitions).

v2: k/v computed first so attention overlaps the q QKV tail; attention is
quarter-major (512-wide i windows) with uniform [128,512] PSUM tiles and
the partial output projection interleaved per quarter; softmax normalize
uses a single tensor_tensor divide; RoPE adds run on idle GpSimd.
"""

import sys
import math

for _p in ("/opt/trn_rl_repo", "/root/.axon_site/_ro/trn_rl_repo"):
    if _p not in sys.path:
        sys.path.insert(0, _p)

import numpy as np
import ml_dtypes

import concourse.bass as bass  # noqa: F401  (registers engine classes)
import concourse.bacc as bacc
import concourse.tile as tile
from concourse import mybir
from concourse.bass_utils import run_bass_kernel_spmd
from concourse.masks import make_identity
from contextlib import ExitStack

BF16 = ml_dtypes.bfloat16
P = 128
T = 2048
C = 2048
NT = T // P        # 16 t-blocks
NCC = C // P       # 16 contraction chunks
NF = 6             # f-blocks per core: q0..q3, k, v
NQ = 4             # query heads per core
FQKV = NF * P      # 768
FY = NQ * P        # 512
SCALE = 1.0 / math.sqrt(P)
NEG = -1.0e30

dt = mybir.dt
AF = mybir.ActivationFunctionType
ALU = mybir.AluOpType

TRACE = False
_CACHE = {}


def _build():
    nc = bacc.Bacc("TRN2", target_bir_lowering=False, debug=False, num_devices=8)
    xT_d = nc.dram_tensor("xT", [C, T], dt.bfloat16, kind="ExternalInput").ap()
    wqkT_d = nc.dram_tensor("wqkT", [C, FQKV], dt.bfloat16, kind="ExternalInput").ap()
    wpT_d = nc.dram_tensor("wpT", [FY, T], dt.bfloat16, kind="ExternalInput").ap()
    cosT_d = nc.dram_tensor("cosT", [P, T], dt.float32, kind="ExternalInput").ap()
    sinS_d = nc.dram_tensor("sinS", [P, T], dt.float32, kind="ExternalInput").ap()
    out_d = nc.dram_tensor("out", [T, C], dt.float32, kind="ExternalOutput").ap()

    with tile.TileContext(nc) as tc, ExitStack() as ctx:
        const = ctx.enter_context(tc.tile_pool(name="const", bufs=1))
        identity = const.tile([P, P], dt.bfloat16, tag="id", name="identity")
        make_identity(nc, identity)
        ones_bf = const.tile([P, P], dt.bfloat16, tag="ones", name="ones_bf")
        nc.gpsimd.memset(ones_bf, 1.0)
        # causal mask for the diagonal 128x128 block of scores^T:
        # element (p=j, f=i): keep 0 where i - j >= 0, else -1e30
        maskf = const.tile([P, P], dt.float32, tag="mask", name="maskf")
        nc.gpsimd.memset(maskf, 0.0)
        nc.gpsimd.affine_select(
            out=maskf, in_=maskf, compare_op=ALU.is_ge, fill=NEG,
            base=0, pattern=[[1, P]], channel_multiplier=-1,
        )

        trig = ctx.enter_context(tc.tile_pool(name="trig", bufs=1))
        cosT = trig.tile([P, T], dt.float32, tag="cos", name="cosT")
        sinS = trig.tile([P, T], dt.float32, tag="sin", name="sinS")

        persist = ctx.enter_context(tc.tile_pool(name="persist", bufs=1))
        qrot = [persist.tile([P, T], dt.bfloat16, tag=f"q{h}", name=f"q{h}") for h in range(NQ)]
        krot = persist.tile([P, T], dt.bfloat16, tag="k", name="krot")
        vraw = persist.tile([P, T], dt.bfloat16, tag="vr", name="vraw")   # v^T (d-major)
        vt = persist.tile([P, T], dt.bfloat16, tag="vt", name="vt")       # v t-major blocks
        y_sb = [persist.tile([P, T], dt.bfloat16, tag=f"y{h}", name=f"ysb{h}") for h in range(NQ)]
        wp_t = [persist.tile([P, T], dt.bfloat16, tag=f"wp{j}", name=f"wp{j}") for j in range(NQ)]

        # DMA order matters for the pipeline head: interleave w/x chunk pairs
        # so the first accumulation chain can start immediately; everything
        # not needed until RoPE / proj loads afterwards.
        xw_pool = ctx.enter_context(tc.tile_pool(name="xw", bufs=1))
        xt, wt = [], []
        for ci in range(NCC):
            tw = xw_pool.tile([P, FQKV], dt.bfloat16, tag=f"w{ci}", name=f"wt{ci}")
            nc.sync.dma_start(tw, wqkT_d[ci * P:(ci + 1) * P, :])
            wt.append(tw)
            tx = xw_pool.tile([P, T], dt.bfloat16, tag=f"x{ci}", name=f"xt{ci}")
            nc.sync.dma_start(tx, xT_d[ci * P:(ci + 1) * P, :])
            xt.append(tx)
        nc.sync.dma_start(cosT, cosT_d)
        nc.sync.dma_start(sinS, sinS_d)
        for j in range(NQ):
            nc.sync.dma_start(wp_t[j], wpT_d[j * P:(j + 1) * P, :])

        # ---------------- Phase 1: QKV^T = wqkT.T @ xT, fused RoPE ----------
        # k and v first so attention can start while q1..q3 still project.
        with tc.tile_pool(name="rtmp", bufs=6) as rtmp, \
             tc.tile_pool(name="qkvps", bufs=6, space="PSUM") as qkvps, \
             tc.tile_pool(name="vtps", bufs=2, space="PSUM") as vtps:
            for f in (4, 5, 0, 1, 2, 3):
                for t4 in (3, 2, 1, 0):  # 512-wide t quarters, one PSUM bank each
                    ps = qkvps.tile([P, 512], dt.float32, tag="qkv", name="qkvps_t")
                    st = slice(t4 * 512, (t4 + 1) * 512)
                    for ci in range(NCC):
                        nc.tensor.matmul(
                            ps,
                            lhsT=wt[ci][:, f * P:(f + 1) * P],
                            rhs=xt[ci][:, st],
                            start=(ci == 0), stop=(ci == NCC - 1),
                        )
                    if f != 5:
                        # RoPE (rotate-halves) in fp32, write bf16
                        dest = qrot[f] if f < NQ else krot
                        t1 = rtmp.tile([P, 512], dt.float32, tag="r1", name="ropet1")
                        nc.vector.tensor_mul(t1, ps, cosT[:, st])
                        t2 = rtmp.tile([P, 512], dt.float32, tag="r2", name="ropet2")
                        nc.vector.tensor_mul(t2[0:64, :], ps[64:128, :], sinS[0:64, st])
                        nc.vector.tensor_mul(t2[64:128, :], ps[0:64, :], sinS[64:128, st])
                        nc.gpsimd.tensor_add(dest[:, st], t1, t2)
                    else:
                        nc.any.tensor_copy(vraw[:, st], ps)
                if f == 5:
                    # v^T -> v (t-major [j-part, d]) via PE transpose
                    for tb in range(NT):
                        pst = vtps.tile([P, P], dt.bfloat16, tag="vtp", name="vtpst")
                        nc.tensor.transpose(pst, vraw[:, tb * P:(tb + 1) * P], identity)
                        nc.any.tensor_copy(vt[:, tb * P:(tb + 1) * P], pst)

        # ------------- Phase 2: attention + interleaved partial proj --------
        # Quarter-major: for each 512-wide i-window, run all 4 heads' causal
        # attention (scores^T chunks [j-part, i-free], ACT exp, y^T and
        # broadcast row-sums via PE), then immediately project those 4
        # t-blocks. Uniform [128,512] PSUM tiles keep all pools in 8 banks.
        with tc.tile_pool(name="strip", bufs=8) as strip_pool, \
             tc.tile_pool(name="ssb", bufs=3) as ssb_pool, \
             tc.tile_pool(name="ostage", bufs=4) as ostage, \
             tc.tile_pool(name="scps", bufs=2, space="PSUM") as scps, \
             tc.tile_pool(name="ypsp", bufs=3, space="PSUM") as ypsp, \
             tc.tile_pool(name="spsp", bufs=1, space="PSUM") as spsp, \
             tc.tile_pool(name="prps", bufs=1, space="PSUM") as prps:
            for q in (3, 2, 1, 0):
                q_lo = q * 512
                for h in range(NQ):
                    qT = qrot[h]
                    yps = ypsp.tile([P, 512], dt.float32, tag="y", name="ypst")
                    sps = spsp.tile([P, 512], dt.float32, tag="s", name="spst")
                    njb = 4 * q + 4
                    for jb in range(njb):
                        j_sl = slice(jb * P, (jb + 1) * P)
                        i_lo = max(jb * P, q_lo)
                        w = q_lo + 512 - i_lo
                        c0 = 512 - w  # column offset inside the 512 window
                        strip = strip_pool.tile([P, 512], dt.bfloat16, tag="strip", name="stript")
                        ps = scps.tile([P, 512], dt.float32, tag="sc", name="scpst")
                        nc.tensor.matmul(
                            ps[:, :w], lhsT=krot[:, j_sl], rhs=qT[:, i_lo:i_lo + w],
                            start=True, stop=True,
                        )
                        if jb >= 4 * q:  # diagonal block: apply causal mask
                            nc.vector.tensor_add(ps[:, :P], ps[:, :P], maskf)
                        nc.scalar.activation(strip[:, :w], ps[:, :w], AF.Exp, scale=SCALE)
                        st_flag = (jb == 0)
                        sp_flag = (jb == njb - 1)
                        nc.tensor.matmul(
                            yps[:, c0:], lhsT=vt[:, j_sl], rhs=strip[:, :w],
                            start=st_flag, stop=sp_flag,
                        )
                        nc.tensor.matmul(
                            sps[:, c0:], lhsT=ones_bf, rhs=strip[:, :w],
                            start=st_flag, stop=sp_flag,
                        )
                    # normalize: y * (1/rowsum) (sums broadcast on all partitions)
                    rcp = ssb_pool.tile([P, 512], dt.float32, tag="ssb", name="rcpt")
                    nc.vector.reciprocal_approx_fast(out=rcp, in_=sps)
                    nc.vector.tensor_mul(y_sb[h][:, q_lo:q_lo + 512], yps, rcp)
                # partial proj for this quarter's 4 t-blocks
                for tb in range(4 * q, 4 * q + 4):
                    t_sl = slice(tb * P, (tb + 1) * P)
                    for oh in range(2):
                        pp = prps.tile([P, 1024], dt.float32, tag="pr", name="prpst")
                        for f4 in range(NQ):
                            for o2 in range(2):
                                o_lo = oh * 1024 + o2 * 512
                                nc.tensor.matmul(
                                    pp[:, o2 * 512:(o2 + 1) * 512],
                                    lhsT=y_sb[f4][:, t_sl],
                                    rhs=wp_t[f4][:, o_lo:o_lo + 512],
                                    start=(f4 == 0), stop=(f4 == NQ - 1),
                                )
                        ot = ostage.tile([P, 1024], dt.float32, tag="o", name="otile")
                        nc.any.tensor_copy(ot, pp)
                        nc.sync.dma_start(out_d[t_sl, oh * 1024:(oh + 1) * 1024], ot)

    nc.compile()
    return nc


def kernel(x, w_attn, w_proj, cos, sin):
    x = np.asarray(x, dtype=np.float32)
    w_attn = np.asarray(w_attn, dtype=np.float32)
    w_proj = np.asarray(w_proj, dtype=np.float32)
    cos = np.asarray(cos, dtype=np.float32)
    sin = np.asarray(sin, dtype=np.float32)

    if "nc" not in _CACHE:
        _CACHE["nc"] = _build()
    nc = _CACHE["nc"]

    cosT = np.ascontiguousarray(cos.T)                      # [128, T] f32
    sinT = np.ascontiguousarray(sin.T)
    sinS = sinT.copy()
    sinS[:64] = -sinS[:64]

    in_maps = []
    for core in range(8):
        b, g = core // 4, core % 4
        xT = np.ascontiguousarray(x[b].T).astype(BF16)                        # [C, T]
        wqkT = np.ascontiguousarray(w_attn[g * FQKV:(g + 1) * FQKV].T).astype(BF16)  # [C, 768]
        wpT = np.ascontiguousarray(w_proj[:, g * FY:(g + 1) * FY].T).astype(BF16)    # [512, T]
        in_maps.append({"xT": xT, "wqkT": wqkT, "wpT": wpT, "cosT": cosT, "sinS": sinS})

    res = run_bass_kernel_spmd(nc, in_maps, core_ids=list(range(8)), trace=TRACE)
    if TRACE:
        _CACHE["last_results"] = res

    out = np.zeros((2, T, C), dtype=np.float32)
    for core in range(8):
        b = core // 4
        out[b] += res.results[core]["out"]
    return out


# revision 15
# speedup vs baseline: 1.0452x; 1.0452x over previous
"""Causal GQA self-attention (B=2, T=2048, C=2048, 16 heads / 4 KV groups,
head_size=128, RoPE) on 8 Trainium2 NeuronCores.

Sharding: tensor-parallel over the 4 KV groups x data-parallel over the 2
batch elements -> 8 cores, core = b*4 + g. Each core computes its group's
QKV projection, RoPE, causal SDPA for the group's 4 query heads, and the
partial output projection (w_proj input-dim shard). The proj partials are
reduced on the host (equivalent of the post-proj all-reduce).

All matmuls run in bf16 with fp32 PSUM accumulation. Inputs are transposed
and cast to bf16 on the host so every DMA is a contiguous, layout-perfect
load (contraction dims land on SBUF partitions).

v2: k/v computed first so attention overlaps the q QKV tail; attention is
quarter-major (512-wide i windows) with uniform [128,512] PSUM tiles and
the partial output projection interleaved per quarter; softmax normalize
uses a single tensor_tensor divide; RoPE adds run on idle GpSimd.
"""

import sys
import math

for _p in ("/opt/trn_rl_repo", "/root/.axon_site/_ro/trn_rl_repo"):
    if _p not in sys.path:
        sys.path.insert(0, _p)

import numpy as np
import ml_dtypes

import concourse.bass as bass  # noqa: F401  (registers engine classes)
import concourse.bacc as bacc
import concourse.tile as tile
from concourse import mybir
from concourse.bass_utils import run_bass_kernel_spmd
from concourse.masks import make_identity
from contextlib import ExitStack

BF16 = ml_dtypes.bfloat16
P = 128
T = 2048
C = 2048
NT = T // P        # 16 t-blocks
NCC = C // P       # 16 contraction chunks
NF = 6             # f-blocks per core: q0..q3, k, v
NQ = 4             # query heads per core
FQKV = NF * P      # 768
FY = NQ * P        # 512
SCALE = 1.0 / math.sqrt(P)
NEG = -1.0e30

dt = mybir.dt
AF = mybir.ActivationFunctionType
ALU = mybir.AluOpType

TRACE = False
_CACHE = {}


def _build():
    nc = bacc.Bacc("TRN2", target_bir_lowering=False, debug=False, num_devices=8)
    xT_d = nc.dram_tensor("xT", [C, T], dt.bfloat16, kind="ExternalInput").ap()
    wqkT_d = nc.dram_tensor("wqkT", [C, FQKV], dt.bfloat16, kind="ExternalInput").ap()
    wpT_d = nc.dram_tensor("wpT", [FY, T], dt.bfloat16, kind="ExternalInput").ap()
    cosT_d = nc.dram_tensor("cosT", [P, T], dt.float32, kind="ExternalInput").ap()
    sinS_d = nc.dram_tensor("sinS", [P, T], dt.float32, kind="ExternalInput").ap()
    out_d = nc.dram_tensor("out", [T, C], dt.float32, kind="ExternalOutput").ap()

    with tile.TileContext(nc) as tc, ExitStack() as ctx:
        const = ctx.enter_context(tc.tile_pool(name="const", bufs=1))
        identity = const.tile([P, P], dt.bfloat16, tag="id", name="identity")
        make_identity(nc, identity)
        ones_bf = const.tile([P, P], dt.bfloat16, tag="ones", name="ones_bf")
        nc.gpsimd.memset(ones_bf, 1.0)
        # causal mask for the diagonal 128x128 block of scores^T:
        # element (p=j, f=i): keep 0 where i - j >= 0, else -1e30
        maskf = const.tile([P, P], dt.float32, tag="mask", name="maskf")
        nc.gpsimd.memset(maskf, 0.0)
        nc.gpsimd.affine_select(
            out=maskf, in_=maskf, compare_op=ALU.is_ge, fill=NEG,
            base=0, pattern=[[1, P]], channel_multiplier=-1,
        )

        trig = ctx.enter_context(tc.tile_pool(name="trig", bufs=1))
        cosT = trig.tile([P, T], dt.float32, tag="cos", name="cosT")
        sinS = trig.tile([P, T], dt.float32, tag="sin", name="sinS")

        persist = ctx.enter_context(tc.tile_pool(name="persist", bufs=1))
        qrot = [persist.tile([P, T], dt.bfloat16, tag=f"q{h}", name=f"q{h}") for h in range(NQ)]
        krot = persist.tile([P, T], dt.bfloat16, tag="k", name="krot")
        vraw = persist.tile([P, T], dt.bfloat16, tag="vr", name="vraw")   # v^T (d-major)
        vt = persist.tile([P, T], dt.bfloat16, tag="vt", name="vt")       # v t-major blocks
        y_sb = [persist.tile([P, T], dt.bfloat16, tag=f"y{h}", name=f"ysb{h}") for h in range(NQ)]
        wp_t = [persist.tile([P, T], dt.bfloat16, tag=f"wp{j}", name=f"wp{j}") for j in range(NQ)]

        # DMA order matters for the pipeline head: interleave w/x chunk pairs
        # so the first accumulation chain can start immediately; everything
        # not needed until RoPE / proj loads afterwards.
        xw_pool = ctx.enter_context(tc.tile_pool(name="xw", bufs=1))
        xt, wt = [], []
        for ci in range(NCC):
            tw = xw_pool.tile([P, FQKV], dt.bfloat16, tag=f"w{ci}", name=f"wt{ci}")
            nc.scalar.dma_start(tw, wqkT_d[ci * P:(ci + 1) * P, :])
            wt.append(tw)
            tx = xw_pool.tile([P, T], dt.bfloat16, tag=f"x{ci}", name=f"xt{ci}")
            nc.sync.dma_start(tx, xT_d[ci * P:(ci + 1) * P, :])
            xt.append(tx)
        nc.scalar.dma_start(cosT, cosT_d)
        nc.scalar.dma_start(sinS, sinS_d)
        for j in range(NQ):
            nc.sync.dma_start(wp_t[j], wpT_d[j * P:(j + 1) * P, :])

        # ---------------- Phase 1: QKV^T = wqkT.T @ xT, fused RoPE ----------
        # k and v first so attention can start while q1..q3 still project.
        with tc.tile_pool(name="rtmp", bufs=6) as rtmp, \
             tc.tile_pool(name="qkvps", bufs=6, space="PSUM") as qkvps, \
             tc.tile_pool(name="vtps", bufs=2, space="PSUM") as vtps:
            for f in (4, 5, 0, 1, 2, 3):
                for t4 in (3, 2, 1, 0):  # 512-wide t quarters, one PSUM bank each
                    ps = qkvps.tile([P, 512], dt.float32, tag="qkv", name="qkvps_t")
                    st = slice(t4 * 512, (t4 + 1) * 512)
                    for ci in range(NCC):
                        nc.tensor.matmul(
                            ps,
                            lhsT=wt[ci][:, f * P:(f + 1) * P],
                            rhs=xt[ci][:, st],
                            start=(ci == 0), stop=(ci == NCC - 1),
                        )
                    if f != 5:
                        # RoPE (rotate-halves) in fp32, write bf16
                        dest = qrot[f] if f < NQ else krot
                        t1 = rtmp.tile([P, 512], dt.float32, tag="r1", name="ropet1")
                        nc.vector.tensor_mul(t1, ps, cosT[:, st])
                        t2 = rtmp.tile([P, 512], dt.float32, tag="r2", name="ropet2")
                        nc.vector.tensor_mul(t2[0:64, :], ps[64:128, :], sinS[0:64, st])
                        nc.vector.tensor_mul(t2[64:128, :], ps[0:64, :], sinS[64:128, st])
                        nc.gpsimd.tensor_add(dest[:, st], t1, t2)
                    else:
                        nc.any.tensor_copy(vraw[:, st], ps)
                if f == 5:
                    # v^T -> v (t-major [j-part, d]) via PE transpose
                    for tb in range(NT):
                        pst = vtps.tile([P, P], dt.bfloat16, tag="vtp", name="vtpst")
                        nc.tensor.transpose(pst, vraw[:, tb * P:(tb + 1) * P], identity)
                        nc.any.tensor_copy(vt[:, tb * P:(tb + 1) * P], pst)

        # ------------- Phase 2: attention + interleaved partial proj --------
        # Quarter-major: for each 512-wide i-window, run all 4 heads' causal
        # attention (scores^T chunks [j-part, i-free], ACT exp, y^T and
        # broadcast row-sums via PE), then immediately project those 4
        # t-blocks. Uniform [128,512] PSUM tiles keep all pools in 8 banks.
        with tc.tile_pool(name="strip", bufs=8) as strip_pool, \
             tc.tile_pool(name="ssb", bufs=3) as ssb_pool, \
             tc.tile_pool(name="ostage", bufs=4) as ostage, \
             tc.tile_pool(name="scps", bufs=3, space="PSUM") as scps, \
             tc.tile_pool(name="ypsp", bufs=2, space="PSUM") as ypsp, \
             tc.tile_pool(name="spsp", bufs=1, space="PSUM") as spsp, \
             tc.tile_pool(name="prps", bufs=1, space="PSUM") as prps:
            for q in (3, 2, 1, 0):
                q_lo = q * 512
                for h in range(NQ):
                    qT = qrot[h]
                    yps = ypsp.tile([P, 512], dt.float32, tag="y", name="ypst")
                    sps = spsp.tile([P, 512], dt.float32, tag="s", name="spst")
                    njb = 4 * q + 4
                    for jb in range(njb):
                        j_sl = slice(jb * P, (jb + 1) * P)
                        i_lo = max(jb * P, q_lo)
                        w = q_lo + 512 - i_lo
                        c0 = 512 - w  # column offset inside the 512 window
                        strip = strip_pool.tile([P, 512], dt.bfloat16, tag="strip", name="stript")
                        ps = scps.tile([P, 512], dt.float32, tag="sc", name="scpst")
                        nc.tensor.matmul(
                            ps[:, :w], lhsT=krot[:, j_sl], rhs=qT[:, i_lo:i_lo + w],
                            start=True, stop=True,
                        )
                        if jb >= 4 * q:  # diagonal block: apply causal mask
                            nc.vector.tensor_add(ps[:, :P], ps[:, :P], maskf)
                        nc.scalar.activation(strip[:, :w], ps[:, :w], AF.Exp, scale=SCALE)
                        st_flag = (jb == 0)
                        sp_flag = (jb == njb - 1)
                        nc.tensor.matmul(
                            yps[:, c0:], lhsT=vt[:, j_sl], rhs=strip[:, :w],
                            start=st_flag, stop=sp_flag,
                        )
                        nc.tensor.matmul(
                            sps[:, c0:], lhsT=ones_bf, rhs=strip[:, :w],
                            start=st_flag, stop=sp_flag,
                        )
                    # normalize: y * (1/rowsum) (sums broadcast on all partitions)
                    rcp = ssb_pool.tile([P, 512], dt.float32, tag="ssb", name="rcpt")
                    nc.vector.reciprocal_approx_fast(out=rcp, in_=sps)
                    nc.vector.tensor_mul(y_sb[h][:, q_lo:q_lo + 512], yps, rcp)
                # partial proj for this quarter's 4 t-blocks
                for tb in range(4 * q, 4 * q + 4):
                    t_sl = slice(tb * P, (tb + 1) * P)
                    for oh in range(2):
                        pp = prps.tile([P, 1024], dt.float32, tag="pr", name="prpst")
                        for f4 in range(NQ):
                            for o2 in range(2):
                                o_lo = oh * 1024 + o2 * 512
                                nc.tensor.matmul(
                                    pp[:, o2 * 512:(o2 + 1) * 512],
                                    lhsT=y_sb[f4][:, t_sl],
                                    rhs=wp_t[f4][:, o_lo:o_lo + 512],
                                    start=(f4 == 0), stop=(f4 == NQ - 1),
                                )
                        ot = ostage.tile([P, 1024], dt.float32, tag="o", name="otile")
                        nc.any.tensor_copy(ot, pp)
                        nc.sync.dma_start(out_d[t_sl, oh * 1024:(oh + 1) * 1024], ot)

    nc.compile()
    return nc


def kernel(x, w_attn, w_proj, cos, sin):
    x = np.asarray(x, dtype=np.float32)
    w_attn = np.asarray(w_attn, dtype=np.float32)
    w_proj = np.asarray(w_proj, dtype=np.float32)
    cos = np.asarray(cos, dtype=np.float32)
    sin = np.asarray(sin, dtype=np.float32)

    if "nc" not in _CACHE:
        _CACHE["nc"] = _build()
    nc = _CACHE["nc"]

    cosT = np.ascontiguousarray(cos.T)                      # [128, T] f32
    sinT = np.ascontiguousarray(sin.T)
    sinS = sinT.copy()
    sinS[:64] = -sinS[:64]

    in_maps = []
    for core in range(8):
        b, g = core // 4, core % 4
        xT = np.ascontiguousarray(x[b].T).astype(BF16)                        # [C, T]
        wqkT = np.ascontiguousarray(w_attn[g * FQKV:(g + 1) * FQKV].T).astype(BF16)  # [C, 768]
        wpT = np.ascontiguousarray(w_proj[:, g * FY:(g + 1) * FY].T).astype(BF16)    # [512, T]
        in_maps.append({"xT": xT, "wqkT": wqkT, "wpT": wpT, "cosT": cosT, "sinS": sinS})

    res = run_bass_kernel_spmd(nc, in_maps, core_ids=list(range(8)), trace=TRACE)
    if TRACE:
        _CACHE["last_results"] = res

    out = np.zeros((2, T, C), dtype=np.float32)
    for core in range(8):
        b = core // 4
        out[b] += res.results[core]["out"]
    return out


# revision 16
# speedup vs baseline: 1.0920x; 1.0448x over previous
"""Causal GQA self-attention (B=2, T=2048, C=2048, 16 heads / 4 KV groups,
head_size=128, RoPE) on 8 Trainium2 NeuronCores.

Sharding: tensor-parallel over the 4 KV groups x data-parallel over the 2
batch elements -> 8 cores, core = b*4 + g. Each core computes its group's
QKV projection, RoPE, causal SDPA for the group's 4 query heads, and the
partial output projection (w_proj input-dim shard). The proj partials are
reduced on the host (equivalent of the post-proj all-reduce).

All matmuls run in bf16 with fp32 PSUM accumulation. Inputs are transposed
and cast to bf16 on the host so every DMA is a contiguous, layout-perfect
load (contraction dims land on SBUF partitions).

v2: k/v computed first so attention overlaps the q QKV tail; attention is
quarter-major (512-wide i windows) with uniform [128,512] PSUM tiles and
the partial output projection interleaved per quarter; softmax normalize
uses a single tensor_tensor divide; RoPE adds run on idle GpSimd.
"""

import sys
import math

for _p in ("/opt/trn_rl_repo", "/root/.axon_site/_ro/trn_rl_repo"):
    if _p not in sys.path:
        sys.path.insert(0, _p)

import numpy as np
import ml_dtypes

import concourse.bass as bass  # noqa: F401  (registers engine classes)
import concourse.bacc as bacc
import concourse.tile as tile
from concourse import mybir
from concourse.bass_utils import run_bass_kernel_spmd
from concourse.masks import make_identity
from contextlib import ExitStack

BF16 = ml_dtypes.bfloat16
P = 128
T = 2048
C = 2048
NT = T // P        # 16 t-blocks
NCC = C // P       # 16 contraction chunks
NF = 6             # f-blocks per core: q0..q3, k, v
NQ = 4             # query heads per core
FQKV = NF * P      # 768
FY = NQ * P        # 512
SCALE = 1.0 / math.sqrt(P)
NEG = -1.0e30

dt = mybir.dt
AF = mybir.ActivationFunctionType
ALU = mybir.AluOpType

TRACE = False
_CACHE = {}


def _build():
    nc = bacc.Bacc("TRN2", target_bir_lowering=False, debug=False, num_devices=8)
    xT_d = nc.dram_tensor("xT", [C, T], dt.bfloat16, kind="ExternalInput").ap()
    wqkT_d = nc.dram_tensor("wqkT", [C, FQKV], dt.bfloat16, kind="ExternalInput").ap()
    wpT_d = nc.dram_tensor("wpT", [FY, T], dt.bfloat16, kind="ExternalInput").ap()
    cosT_d = nc.dram_tensor("cosT", [P, T], dt.float32, kind="ExternalInput").ap()
    sinS_d = nc.dram_tensor("sinS", [P, T], dt.float32, kind="ExternalInput").ap()
    out_d = nc.dram_tensor("out", [T, C], dt.float32, kind="ExternalOutput").ap()

    with tile.TileContext(nc) as tc, ExitStack() as ctx:
        const = ctx.enter_context(tc.tile_pool(name="const", bufs=1))
        identity = const.tile([P, P], dt.bfloat16, tag="id", name="identity")
        make_identity(nc, identity)
        ones_bf = const.tile([P, P], dt.bfloat16, tag="ones", name="ones_bf")
        nc.gpsimd.memset(ones_bf, 1.0)
        # causal mask for the diagonal 128x128 block of scores^T:
        # element (p=j, f=i): keep 0 where i - j >= 0, else -1e30
        maskf = const.tile([P, P], dt.float32, tag="mask", name="maskf")
        nc.gpsimd.memset(maskf, 0.0)
        nc.gpsimd.affine_select(
            out=maskf, in_=maskf, compare_op=ALU.is_ge, fill=NEG,
            base=0, pattern=[[1, P]], channel_multiplier=-1,
        )

        trig = ctx.enter_context(tc.tile_pool(name="trig", bufs=1))
        cosT = trig.tile([P, T], dt.float32, tag="cos", name="cosT")
        sinS = trig.tile([P, T], dt.float32, tag="sin", name="sinS")

        persist = ctx.enter_context(tc.tile_pool(name="persist", bufs=1))
        qrot = [persist.tile([P, T], dt.bfloat16, tag=f"q{h}", name=f"q{h}") for h in range(NQ)]
        krot = persist.tile([P, T], dt.bfloat16, tag="k", name="krot")
        vraw = persist.tile([P, T], dt.bfloat16, tag="vr", name="vraw")   # v^T (d-major)
        vt = persist.tile([P, T], dt.bfloat16, tag="vt", name="vt")       # v t-major blocks
        y_sb = [persist.tile([P, T], dt.bfloat16, tag=f"y{h}", name=f"ysb{h}") for h in range(NQ)]
        wp_t = [persist.tile([P, T], dt.bfloat16, tag=f"wp{j}", name=f"wp{j}") for j in range(NQ)]

        # DMA order matters for the pipeline head: interleave w/x chunk pairs
        # so the first accumulation chain can start immediately; everything
        # not needed until RoPE / proj loads afterwards.
        xw_pool = ctx.enter_context(tc.tile_pool(name="xw", bufs=1))
        xt, wt = [], []
        for ci in range(NCC):
            tw = xw_pool.tile([P, FQKV], dt.bfloat16, tag=f"w{ci}", name=f"wt{ci}")
            nc.sync.dma_start(tw, wqkT_d[ci * P:(ci + 1) * P, :])
            wt.append(tw)
            tx = xw_pool.tile([P, T], dt.bfloat16, tag=f"x{ci}", name=f"xt{ci}")
            nc.sync.dma_start(tx, xT_d[ci * P:(ci + 1) * P, :])
            xt.append(tx)
        nc.sync.dma_start(cosT, cosT_d)
        nc.sync.dma_start(sinS, sinS_d)
        for j in range(NQ):
            nc.sync.dma_start(wp_t[j], wpT_d[j * P:(j + 1) * P, :])

        # ---------------- Phase 1: QKV^T = wqkT.T @ xT, fused RoPE ----------
        # k and v first so attention can start while q1..q3 still project.
        with tc.tile_pool(name="rtmp", bufs=6) as rtmp, \
             tc.tile_pool(name="qkvps", bufs=7, space="PSUM") as qkvps, \
             tc.tile_pool(name="vtps", bufs=1, space="PSUM") as vtps:
            for f in (4, 5, 0, 1, 2, 3):
                for t4 in (3, 2, 1, 0):  # 512-wide t quarters, one PSUM bank each
                    ps = qkvps.tile([P, 512], dt.float32, tag="qkv", name="qkvps_t")
                    st = slice(t4 * 512, (t4 + 1) * 512)
                    for ci in range(NCC):
                        nc.tensor.matmul(
                            ps,
                            lhsT=wt[ci][:, f * P:(f + 1) * P],
                            rhs=xt[ci][:, st],
                            start=(ci == 0), stop=(ci == NCC - 1),
                        )
                    if f != 5:
                        # RoPE (rotate-halves) in fp32, write bf16
                        dest = qrot[f] if f < NQ else krot
                        t1 = rtmp.tile([P, 512], dt.float32, tag="r1", name="ropet1")
                        nc.vector.tensor_mul(t1, ps, cosT[:, st])
                        t2 = rtmp.tile([P, 512], dt.float32, tag="r2", name="ropet2")
                        nc.vector.tensor_mul(t2[0:64, :], ps[64:128, :], sinS[0:64, st])
                        nc.vector.tensor_mul(t2[64:128, :], ps[0:64, :], sinS[64:128, st])
                        nc.gpsimd.tensor_add(dest[:, st], t1, t2)
                    else:
                        nc.any.tensor_copy(vraw[:, st], ps)
                if f == 5:
                    # v^T -> v (t-major [j-part, d]) via PE transpose
                    for tb in range(NT):
                        pst = vtps.tile([P, P], dt.bfloat16, tag="vtp", name="vtpst")
                        nc.tensor.transpose(pst, vraw[:, tb * P:(tb + 1) * P], identity)
                        nc.any.tensor_copy(vt[:, tb * P:(tb + 1) * P], pst)

        # ------------- Phase 2: attention + interleaved partial proj --------
        # Quarter-major: for each 512-wide i-window, run all 4 heads' causal
        # attention (scores^T chunks [j-part, i-free], ACT exp, y^T and
        # broadcast row-sums via PE), then immediately project those 4
        # t-blocks. Uniform [128,512] PSUM tiles keep all pools in 8 banks.
        with tc.tile_pool(name="strip", bufs=8) as strip_pool, \
             tc.tile_pool(name="ssb", bufs=3) as ssb_pool, \
             tc.tile_pool(name="ostage", bufs=4) as ostage, \
             tc.tile_pool(name="scps", bufs=3, space="PSUM") as scps, \
             tc.tile_pool(name="ypsp", bufs=2, space="PSUM") as ypsp, \
             tc.tile_pool(name="spsp", bufs=1, space="PSUM") as spsp, \
             tc.tile_pool(name="prps", bufs=1, space="PSUM") as prps:
            for q in (3, 2, 1, 0):
                q_lo = q * 512
                for h in range(NQ):
                    qT = qrot[h]
                    yps = ypsp.tile([P, 512], dt.float32, tag="y", name="ypst")
                    sps = spsp.tile([P, 512], dt.float32, tag="s", name="spst")
                    njb = 4 * q + 4
                    for jb in range(njb):
                        j_sl = slice(jb * P, (jb + 1) * P)
                        i_lo = max(jb * P, q_lo)
                        w = q_lo + 512 - i_lo
                        c0 = 512 - w  # column offset inside the 512 window
                        strip = strip_pool.tile([P, 512], dt.bfloat16, tag="strip", name="stript")
                        ps = scps.tile([P, 512], dt.float32, tag="sc", name="scpst")
                        nc.tensor.matmul(
                            ps[:, :w], lhsT=krot[:, j_sl], rhs=qT[:, i_lo:i_lo + w],
                            start=True, stop=True,
                        )
                        if jb >= 4 * q:  # diagonal block: apply causal mask
                            nc.vector.tensor_add(ps[:, :P], ps[:, :P], maskf)
                        nc.scalar.activation(strip[:, :w], ps[:, :w], AF.Exp, scale=SCALE)
                        st_flag = (jb == 0)
                        sp_flag = (jb == njb - 1)
                        nc.tensor.matmul(
                            yps[:, c0:], lhsT=vt[:, j_sl], rhs=strip[:, :w],
                            start=st_flag, stop=sp_flag,
                        )
                        nc.tensor.matmul(
                            sps[:, c0:], lhsT=ones_bf, rhs=strip[:, :w],
                            start=st_flag, stop=sp_flag,
                        )
                    # normalize: y * (1/rowsum) (sums broadcast on all partitions)
                    rcp = ssb_pool.tile([P, 512], dt.float32, tag="ssb", name="rcpt")
                    nc.vector.reciprocal_approx_fast(out=rcp, in_=sps)
                    nc.vector.tensor_mul(y_sb[h][:, q_lo:q_lo + 512], yps, rcp)
                # partial proj for this quarter's 4 t-blocks
                for tb in range(4 * q, 4 * q + 4):
                    t_sl = slice(tb * P, (tb + 1) * P)
                    for oh in range(2):
                        pp = prps.tile([P, 1024], dt.float32, tag="pr", name="prpst")
                        for f4 in range(NQ):
                            for o2 in range(2):
                                o_lo = oh * 1024 + o2 * 512
                                nc.tensor.matmul(
                                    pp[:, o2 * 512:(o2 + 1) * 512],
                                    lhsT=y_sb[f4][:, t_sl],
                                    rhs=wp_t[f4][:, o_lo:o_lo + 512],
                                    start=(f4 == 0), stop=(f4 == NQ - 1),
                                )
                        ot = ostage.tile([P, 1024], dt.float32, tag="o", name="otile")
                        nc.any.tensor_copy(ot, pp)
                        nc.sync.dma_start(out_d[t_sl, oh * 1024:(oh + 1) * 1024], ot)

    nc.compile()
    return nc


def kernel(x, w_attn, w_proj, cos, sin):
    x = np.asarray(x, dtype=np.float32)
    w_attn = np.asarray(w_attn, dtype=np.float32)
    w_proj = np.asarray(w_proj, dtype=np.float32)
    cos = np.asarray(cos, dtype=np.float32)
    sin = np.asarray(sin, dtype=np.float32)

    if "nc" not in _CACHE:
        _CACHE["nc"] = _build()
    nc = _CACHE["nc"]

    cosT = np.ascontiguousarray(cos.T)                      # [128, T] f32
    sinT = np.ascontiguousarray(sin.T)
    sinS = sinT.copy()
    sinS[:64] = -sinS[:64]

    in_maps = []
    for core in range(8):
        b, g = core // 4, core % 4
        xT = np.ascontiguousarray(x[b].T).astype(BF16)                        # [C, T]
        wqkT = np.ascontiguousarray(w_attn[g * FQKV:(g + 1) * FQKV].T).astype(BF16)  # [C, 768]
        wpT = np.ascontiguousarray(w_proj[:, g * FY:(g + 1) * FY].T).astype(BF16)    # [512, T]
        in_maps.append({"xT": xT, "wqkT": wqkT, "wpT": wpT, "cosT": cosT, "sinS": sinS})

    res = run_bass_kernel_spmd(nc, in_maps, core_ids=list(range(8)), trace=TRACE)
    if TRACE:
        _CACHE["last_results"] = res

    out = np.zeros((2, T, C), dtype=np.float32)
    for core in range(8):
        b = core // 4
        out[b] += res.results[core]["out"]
    return out
